# revision 22
# baseline (speedup 1.0000x reference)
"""Trainium2 Bass kernel for causal+padded multi-head attention.

Problem: B=2, N=2048, D=1024, H=16 heads (DK=64), fp32 I/O.
  out = softmax(mask(x Wq^T (x Wk^T)^T) / sqrt(DK)) (x Wv^T) Wout^T + b_out

Sharding (8 cores): core c handles batch b=c//4 and heads [4*(c%4), 4*(c%4)+4).
Each core computes a partial output [N, D] (its 4 heads' contribution through
the output projection) in bf16; the host sums the 4 partials per batch and
adds b_out.

v2 schedule: the attention stream (S^T matmul + exp + PV) is ScalarE-bound
(~81us of exp at 1 elem/cycle/lane vs ~44us of PE work), so the projection
matmuls (~55us of PE work) are NOT done in a separate phase: they are emitted
as PE filler thunks interleaved into the attention units, keeping the PE busy
(HAM clock at 2.4 GHz) while ScalarE grinds through the exps.

Per-core layout:
  xT   [1024, 2048]  (host-pretransposed x[b]), loaded per (e,nq) chunk
  QT/KT stored transposed [dk, n] as head-pair tiles [128, 2048]
  V    stored natural as [128(keys), 16 blocks, 4 heads, 65] with a ones
       column appended (col 64) so P@V' also yields the softmax denominator.
  S^T  per (head-pair, q-tile 512, key-block 128) as [128, 2, 512] in PSUM:
       matmul(lhsT=KT slice [64,128], rhs=QT slice [64,512]); the two heads
       sit at base partitions 0/64 so their matmuls row-tile and run
       concurrently on the PE. Causal masking = additive -30000 on PSUM
       (DVE); padding mask is a per-key bias fused into the exp; one
       exp(0.125*s + bias) on ScalarE writes P^T straight to SBUF as bf16.
  ctx'^T [65, 512] accumulated in PSUM over key blocks; PV matmuls are
       emitted one unit behind their exps (riffled into the next unit's S
       stream) so the in-order PE never drains waiting on ScalarE.
  Normalization (one unit behind PV): ctx PSUM rows are copied out on DVE
       immediately after the last PV (so the 2 ctx PSUM banks recycle
       without waiting on the softmax-denominator reciprocal), then
       r = reciprocal_approx_fast(denom row) (single custom-DVE op, ~5x
       faster than the iterative divide), partition-broadcast (GpSimd),
       one DVE multiply into the bf16 ctx buffer.
  Out projection per q-tile once both head-pairs are normalized:
       matmul(lhsT=ctxT [128,128], rhs=WoutT [128,512]) acc over the two
       head-pair chunks, cast bf16, DMA out.

All matmul operands are bf16 (pre-rounded on host for the inputs; on-device
casts for intermediates); accumulation is fp32 in PSUM, and the softmax /
masking / normalization arithmetic is fp32.
"""

import math
import os

import numpy as np

B, N, D, H = 2, 2048, 1024, 16
DK = D // H  # 64
NCORES = 8
HEADS_PER_CORE = 4
QTILE = 512
KBLK = 128
NEG = -30000.0
NEGB = -3750.0  # pad bias applied after the 0.125 scale inside exp
SCALE = 1.0 / math.sqrt(float(DK))  # 0.125

# Set by run() when tracing is enabled (test.py reads this).
LAST_RESULTS = None


def _build_program(kb_max: int, jpad_min: int):
    import concourse.tile as tile
    from concourse import bacc, mybir

    F32 = mybir.dt.float32
    BF16 = mybir.dt.bfloat16
    EXP = mybir.ActivationFunctionType.Exp
    ADD = mybir.AluOpType.add
    MULT = mybir.AluOpType.mult

    nc = bacc.Bacc(None)

    xt_d = nc.dram_tensor("xt", [D, N], BF16, kind="ExternalInput")
    wqkv_d = nc.dram_tensor("wqkv", [D, 768], BF16, kind="ExternalInput")
    wout_d = nc.dram_tensor("wout", [256, D], BF16, kind="ExternalInput")
    ptri_d = nc.dram_tensor("ptri", [128, 912], F32, kind="ExternalInput")
    ones_d = nc.dram_tensor("ones65", [128, 64], BF16, kind="ExternalInput")
    out_d = nc.dram_tensor("out", [N, D], BF16, kind="ExternalOutput")

    NB = N // KBLK  # 16 key/row blocks
    NQT = N // QTILE  # 4 q tiles
    nb_used = min(NB, kb_max)

    with tile.TileContext(nc) as tc:
        with (
            tc.tile_pool(name="w", bufs=1) as w_pool,
            tc.tile_pool(name="big", bufs=1) as big_pool,
            tc.tile_pool(name="craw", bufs=5) as craw_pool,
            tc.tile_pool(name="work", bufs=2) as work_pool,
            tc.tile_pool(name="osb", bufs=7) as osb_pool,
            tc.tile_pool(name="pt", bufs=26) as pt_pool,
            tc.tile_pool(name="ps_main", bufs=2, space="PSUM") as ps_main,
            tc.tile_pool(name="ps_aux", bufs=2, space="PSUM") as ps_aux,
            tc.tile_pool(name="ps_ctx", bufs=1, space="PSUM") as ps_ctx,
        ):
            # ---- input DMAs (DMA queues run independently of engines) ----
            wqkv_t = w_pool.tile([128, 8, 768], BF16, tag="wqkv")
            wo_t = w_pool.tile([128, 2, D], BF16, tag="wo")
            nc.sync.dma_start(
                wqkv_t[:], wqkv_d[:].rearrange("(e p) m -> p e m", p=128)
            )
            # denominator staging: one tile per reciprocal batch (the DVE
            # Reciprocal and the batch source must start at partition 0)
            stages = [
                w_pool.tile([8, 512], F32, tag="stageA", name="stageA"),
                w_pool.tile([4, 512], F32, tag="stageB", name="stageB"),
                w_pool.tile([4, 512], F32, tag="stageC", name="stageC"),
            ]
            rstages = [
                w_pool.tile([8, 512], F32, tag="rstageA", name="rstageA"),
                w_pool.tile([4, 512], F32, tag="rstageB", name="rstageB"),
                w_pool.tile([4, 512], F32, tag="rstageC", name="rstageC"),
            ]

            def stage_slot(uidx, hh):
                if uidx < 4:
                    return 0, 2 * uidx + hh
                if uidx < 6:
                    return 1, 2 * (uidx - 4) + hh
                return 2, 2 * (uidx - 6) + hh

            ptri_t = w_pool.tile([128, 912], F32, tag="ptri")
            padb_t = ptri_t[:, 0:16]
            trineg_t = ptri_t[:, 16:912]
            # xT chunks: [e, nq] -> [128, 512]; issued in the order the
            # units consume them (q-tile 0 then 3 then 1 then 2).  Issues
            # are spread across the Sync / Scalar / Tensor engine queues:
            # each issue costs ~0.6us on its engine and each HW queue only
            # moves ~21 GB/s, so both issue serialization and queue depth
            # matter.  Scalar and Tensor are idle during the lead-in.
            xt = [big_pool.tile([128, N], BF16, tag=f"xt{e}", name=f"xt{e}") for e in range(8)]
            for e in range(8):
                nc.sync.dma_start(
                    xt[e][:, 0:512], xt_d[e * 128:(e + 1) * 128, 0:512]
                )
            nc.sync.dma_start(ptri_t[:], ptri_d[:])
            for nq in (3, 1, 2):
                for e in range(8):
                    nc.sync.dma_start(
                        xt[e][:, nq * 512:(nq + 1) * 512],
                        xt_d[e * 128:(e + 1) * 128, nq * 512:(nq + 1) * 512],
                    )
            nc.sync.dma_start(wo_t[:], wout_d[:].rearrange("(c p) m -> p c m", p=128))

            # V' tile: [keys 128, key-block 16, head 4, 65]; col 64 <- ones
            v4 = big_pool.tile([128, NB, 4, 65], BF16, tag="v4")
            nc.sync.dma_start(
                v4[:, :, :, 64:65],
                ones_d[:].rearrange("p (b h o) -> p b h o", h=4, o=1),
            )

            qt_pair = [big_pool.tile([128, N], BF16, tag=f"qt{p}", name=f"qt{p}") for p in range(2)]
            kt_pair = [big_pool.tile([128, N], BF16, tag=f"kt{p}", name=f"kt{p}") for p in range(2)]
            ctx_pair = [big_pool.tile([128, N], BF16, tag=f"ctx{p}", name=f"ctx{p}") for p in range(2)]

            # ---- projection thunks (PE filler during the attention stream) --
            def emit_qk(name, pair, nq):
                wofs = 0 if name == "q" else 256
                dst = qt_pair[pair] if name == "q" else kt_pair[pair]
                ps = ps_aux.tile([128, 512], F32, tag="aux")
                for e in range(8):
                    nc.tensor.matmul(
                        ps[:],
                        wqkv_t[:, e, wofs + pair * 128:wofs + (pair + 1) * 128],
                        xt[e][:, nq * 512:(nq + 1) * 512],
                        start=(e == 0),
                        stop=(e == 7),
                    )
                nc.vector.tensor_copy(dst[:, nq * 512:(nq + 1) * 512], ps[:])

            def emit_v(m):
                # V natural for key blocks {2m, 2m+1}: [keys, 4*64]
                ps = ps_aux.tile([128, 512], F32, tag="aux")
                for h in range(2):
                    nb = 2 * m + h
                    if nb >= nb_used:
                        continue
                    for e in range(8):
                        nc.tensor.matmul(
                            ps[:, h * 256:(h + 1) * 256],
                            xt[e][:, nb * 128:(nb + 1) * 128],
                            wqkv_t[:, e, 512:768],
                            start=(e == 0),
                            stop=(e == 7),
                        )
                hi = min(2 * m + 2, nb_used)
                nc.vector.tensor_copy(
                    v4[:, 2 * m:hi, :, 0:64],
                    ps[:, 0:(hi - 2 * m) * 256].rearrange(
                        "p (b h d) -> p b h d", h=4, d=64
                    ),
                )

            # ---- attention units ----
            # A unit is (head-pair, q-tile). Ordered so that ScalarE-light
            # units (qt0) come first (lead-in) and heavy/medium alternate.
            units = []
            for pair, qt in [(0, 0), (1, 0), (0, 3), (1, 3),
                             (0, 1), (1, 1), (0, 2), (1, 2)]:
                units.append((pair, qt, min(4 * qt + 4, kb_max)))

            def nqs_needed(qt):
                return min(qt + 1, (kb_max + 3) // 4)

            # filler thunks per unit index (emitted inside the unit's j-loop)
            fillers = [[] for _ in range(9)]
            # prelude (before unit 0): its own QT/KT
            fillers[0] = [lambda: emit_qk("q", 0, 0), lambda: emit_qk("k", 0, 0)]
            fillers[1] = [lambda: emit_qk("q", 1, 0), lambda: emit_qk("k", 1, 0),
                          lambda: emit_v(0), lambda: emit_v(1)]
            fillers[2] = [lambda: emit_qk("q", 0, 3)] + [
                (lambda nq: lambda: emit_qk("k", 0, nq))(nq)
                for nq in range(1, nqs_needed(3))
            ]
            fillers[3] = [lambda: emit_qk("q", 1, 3)] + [
                (lambda nq: lambda: emit_qk("k", 1, nq))(nq)
                for nq in range(1, nqs_needed(3))
            ] + [(lambda m: lambda: emit_v(m))(m)
                 for m in range(2, (nb_used + 1) // 2)]
            fillers[4] = [lambda: emit_qk("q", 0, 1)]
            fillers[5] = [lambda: emit_qk("q", 1, 1)]
            fillers[6] = [lambda: emit_qk("q", 0, 2)]
            fillers[7] = [lambda: emit_qk("q", 1, 2)]
            # fillers[k] emitted during unit k-1's loop (shift by one):
            # fillers[0] is the prelude, emitted before the loop.

            def emit_st_exp(pair, qt, nchunks, prev, fill):
                """S^T + mask + exp for both heads, with the previous unit's
                PV matmuls riffled in and projection/outproj filler thunks
                spread across the j-loop."""
                if prev is None:
                    ppv = []
                else:
                    ppair, pqt, pn, ppv, pctx2 = prev

                def rif(k):
                    while ppv and ppv[0][0] <= k:
                        jj, ptt, poff = ppv.pop(0)
                        for hh in range(2):
                            nc.tensor.matmul(
                                pctx2[hh][:, poff:],
                                v4[:, jj, 2 * ppair + hh, :],
                                ptt[:, hh, poff:],
                                start=(jj == 0),
                                stop=(jj == pn - 1),
                                skip_group_check=True,
                            )

                nfill = len(fill)
                fill_at = [
                    (i * nchunks) // nfill if nfill else 0 for i in range(nfill)
                ]
                fi = 0

                pv = []
                for j in range(nchunks):
                    rif(j)
                    d = j - 4 * qt
                    # exact-causal column trim (keep matmul N >= 256)
                    off = 128 * d if d >= 1 else 0
                    st_ps = ps_main.tile([128, 2, 512], F32, tag="blk")
                    for hh in range(2):
                        hp = slice(64 * hh, 64 * hh + 64)
                        nc.tensor.matmul(
                            st_ps[:, hh, off:],
                            kt_pair[pair][hp, j * 128:(j + 1) * 128],
                            qt_pair[pair][hp, qt * 512 + off:(qt + 1) * 512],
                            start=True,
                            stop=True,
                        )
                    if d >= 0:
                        # causal add -30000; with off = 128*d the masked
                        # triangle lies entirely in cols [off, off+128)
                        u0 = 384 - 128 * d + off
                        w = min(128, 512 - off)
                        for hh in range(2):
                            nc.vector.tensor_tensor(
                                st_ps[:, hh, off:off + w],
                                st_ps[:, hh, off:off + w],
                                trineg_t[:, u0:u0 + w],
                                ADD,
                            )
                    pt_t = pt_pool.tile([128, 2, 512], BF16, tag="pt")
                    kw = {}
                    if j >= jpad_min:  # per-key pad bias (same for both heads)
                        kw["bias"] = padb_t[:, j:j + 1]
                    nc.scalar.activation(
                        pt_t[:, :, off:], st_ps[:, :, off:], EXP, scale=SCALE, **kw
                    )
                    pv.append((j, pt_t, off))
                    while fi < nfill and fill_at[fi] <= j:
                        fill[fi]()
                        fi += 1
                rif(10 ** 9)
                while fi < nfill:
                    fill[fi]()
                    fi += 1
                return pv

            def emit_pv(pair, qt, nchunks, pv, ctx2):
                for j, pt_t, off in pv:
                    for hh in range(2):
                        nc.tensor.matmul(
                            ctx2[hh][:, off:],
                            v4[:, j, 2 * pair + hh, :],
                            pt_t[:, hh, off:],
                            start=(j == 0),
                            stop=(j == nchunks - 1),
                            skip_group_check=True,
                        )

            def emit_copyout(uidx, ctx2):
                # move ctx (incl denominator row) out of PSUM right away so
                # the 2 ctx banks recycle without waiting on normalization;
                # denominator rows are DMA'd onto consecutive partitions of
                # `stage` so one batched reciprocal covers many units.
                craws = []
                for hh in range(2):
                    craw = craw_pool.tile([65, 512], F32, tag=f"craw{hh}", name=f"craw{hh}")
                    nc.vector.tensor_copy(craw[:], ctx2[hh][:])
                    bi, r = stage_slot(uidx, hh)
                    nc.sync.dma_start(stages[bi][r:r + 1, :], craw[64:65, :])
                    craws.append(craw)
                return craws

            def emit_recip(bi):
                nc.vector.reciprocal(rstages[bi][:], stages[bi][:])

            def emit_normalize(uidx, pair, qt, craws, hh):
                hp = slice(64 * hh, 64 * hh + 64)
                craw = craws[hh]
                bi, r = stage_slot(uidx, hh)
                rrow = work_pool.tile([1, 512], F32, tag="rrow")
                nc.sync.dma_start(rrow[:], rstages[bi][r:r + 1, :])
                rbr = work_pool.tile([64, 512], F32, tag="rbr")
                nc.gpsimd.partition_broadcast(rbr[:], rrow[:])
                nc.vector.tensor_tensor(
                    ctx_pair[pair][hp, qt * 512:(qt + 1) * 512],
                    craw[0:64, :],
                    rbr[:],
                    MULT,
                )

            deferred_osb = []

            def emit_outproj_nb(nb, tail=False, defer_dma=False):
                # output projection for one 128-row n-block
                osb = osb_pool.tile([128, D], BF16, tag="osb")
                for fc in range(2):
                    ps = ps_aux.tile([128, 512], F32, tag="aux")
                    for pr2 in range(2):
                        nc.tensor.matmul(
                            ps[:],
                            ctx_pair[pr2][:, nb * 128:(nb + 1) * 128],
                            wo_t[:, pr2, fc * 512:(fc + 1) * 512],
                            start=(pr2 == 0),
                            stop=(pr2 == 1),
                        )
                    if tail:
                        # ScalarE is idle in the tail; DVE is not
                        nc.scalar.copy(osb[:, fc * 512:(fc + 1) * 512], ps[:])
                    else:
                        nc.vector.tensor_copy(
                            osb[:, fc * 512:(fc + 1) * 512], ps[:]
                        )
                if defer_dma:
                    # queue the bulky output DMA later so the tail's tiny
                    # stage/rrow DMAs don't sit behind it on the sync queues
                    deferred_osb.append((nb, osb))
                else:
                    nc.sync.dma_start(out_d[nb * 128:(nb + 1) * 128, :], osb[:])

            # ---- main loop ----
            # per unit k: st_exp(k) [rifles PV(k-1) + filler thunks],
            # copyout(k-1).  Normalization runs in 3 static batches (one
            # batched reciprocal each) riffled into later units' loops:
            #   during u5: recip rows 0:8, norm u0/u1, outproj qt0
            #   during u6: norm u2/u3, outproj qt3
            #   during u7: recip rows 8:12, norm u4/u5, outproj qt1
            #   tail:      recip rows 12:16, norm u6/u7, outproj qt2
            craws_by_unit = {}

            def norm_thunks(uidx):
                pair, qt, _ = units[uidx]
                return [
                    (lambda hh: lambda: emit_normalize(
                        uidx, pair, qt, craws_by_unit[uidx], hh))(hh)
                    for hh in range(2)
                ]

            def op_thunks(nbs, tail=False, defer_dma=False):
                return [
                    (lambda nb: lambda: emit_outproj_nb(
                        nb, tail=tail, defer_dma=defer_dma))(nb)
                    for nb in nbs
                ]

            fillers[6] += ([lambda: emit_recip(0)] + norm_thunks(0)
                           + norm_thunks(1) + op_thunks(range(0, 4)))
            fillers[7] += (norm_thunks(2) + norm_thunks(3)
                           + op_thunks(range(12, 16)))
            fillers[8] += ([lambda: emit_recip(1)] + norm_thunks(4)
                           + norm_thunks(5) + op_thunks(range(4, 8)))

            prev_pv = None
            for k, (pair, qt, nchunks) in enumerate(units):
                if k == 0:
                    # prelude projections must precede the first S matmul
                    for th in fillers[0]:
                        th()
                pv = emit_st_exp(pair, qt, nchunks, prev_pv, fillers[k + 1])
                if prev_pv is not None:
                    craws_by_unit[k - 1] = emit_copyout(k - 1, prev_pv[4])
                ctx2 = [
                    ps_ctx.tile([65, 512], F32, tag=f"ctx{hh}", name=f"ctx{hh}")
                    for hh in range(2)
                ]
                prev_pv = (pair, qt, nchunks, pv, ctx2)

            # ---- tail ----
            ppair, pqt, pn, ppv, pctx2 = prev_pv
            emit_pv(ppair, pqt, pn, ppv, pctx2)
            craws_by_unit[7] = emit_copyout(7, pctx2)
            emit_recip(2)
            for uidx in (6, 7):
                pair, qt, _ = units[uidx]
                for hh in range(2):
                    emit_normalize(uidx, pair, qt, craws_by_unit[uidx], hh)
            for nb, osb in deferred_osb:
                nc.sync.dma_start(out_d[nb * 128:(nb + 1) * 128, :], osb[:])
            deferred_osb.clear()
            for nb in range(8, 12):
                emit_outproj_nb(nb, tail=True)

    nc.compile()
    return nc


_PROGRAM_CACHE = {}


def kernel(x, attention_mask, W_Q, W_K, W_V, W_out, b_out):
    global LAST_RESULTS
    from concourse.bass_utils import run_bass_kernel_spmd

    x = np.ascontiguousarray(x, dtype=np.float32)
    attention_mask = np.asarray(attention_mask)
    lengths = attention_mask.astype(np.int64).sum(axis=1)
    kb_max = int(math.ceil(lengths.max() / KBLK))
    jpad_min = int(lengths.min() // KBLK)

    key = (kb_max, jpad_min)
    if key not in _PROGRAM_CACHE:
        _PROGRAM_CACHE[key] = _build_program(kb_max, jpad_min)
    nc = _PROGRAM_CACHE[key]

    # host-side input prep (matmul operands pre-cast to bf16)
    import ml_dtypes
    BF = ml_dtypes.bfloat16
    xT = [np.ascontiguousarray(x[b].T.astype(BF)) for b in range(B)]
    wqT = np.asarray(W_Q, dtype=np.float32).T.astype(BF)
    wkT = np.asarray(W_K, dtype=np.float32).T.astype(BF)
    wvT = np.asarray(W_V, dtype=np.float32).T.astype(BF)
    woT = np.ascontiguousarray(np.asarray(W_out, dtype=np.float32).T.astype(BF))
    # padbias[p, j] = 0 if key j*128+p is real else -3750; fused with trineg
    # (trineg[p, u] = NEG if u < p + 384 else 0) into one [128, 912] input.
    pp = np.arange(128)[:, None]
    uu = np.arange(896)[None, :]
    trineg = np.where(uu < pp + 384, NEG, 0.0).astype(np.float32)
    ptri = []
    for b in range(B):
        padb = np.where(
            attention_mask[b].reshape(16, 128).T != 0, 0.0, NEGB
        ).astype(np.float32)
        ptri.append(np.ascontiguousarray(np.concatenate([padb, trineg], axis=1)))
    ones65 = np.ones((128, 64), dtype=BF)

    in_maps = []
    for c in range(NCORES):
        b, g = divmod(c, 4)
        sl = slice(g * 256, (g + 1) * 256)
        wqkv = np.ascontiguousarray(
            np.concatenate([wqT[:, sl], wkT[:, sl], wvT[:, sl]], axis=1)
        )
        in_maps.append(
            {
                "xt": xT[b],
                "wqkv": wqkv,
                "wout": np.ascontiguousarray(woT[sl, :]),
                "ptri": ptri[b],
                "ones65": ones65,
            }
        )

    trace = bool(int(os.environ.get("KERNEL_TRACE", "0")))
    ncores_run = int(os.environ.get("KERNEL_NCORES", str(NCORES)))
    res = run_bass_kernel_spmd(
        nc,
        in_maps[:ncores_run],
        core_ids=list(range(ncores_run)),
        trace=trace,
        trace_cores=list(range(ncores_run)) if trace else None,
    )
    LAST_RESULTS = res

    out = np.zeros((B, N, D), dtype=np.float32)
    for c in range(len(res.results)):
        out[c // 4] += np.asarray(res.results[c]["out"], dtype=np.float32)
    out += np.asarray(b_out, dtype=np.float32)[None, None, :]
    return out


# revision 23
# speedup vs baseline: 1.0095x; 1.0095x over previous
"""Trainium2 Bass kernel for causal+padded multi-head attention.

Problem: B=2, N=2048, D=1024, H=16 heads (DK=64), fp32 I/O.
  out = softmax(mask(x Wq^T (x Wk^T)^T) / sqrt(DK)) (x Wv^T) Wout^T + b_out

Sharding (8 cores): core c handles batch b=c//4 and heads [4*(c%4), 4*(c%4)+4).
Each core computes a partial output [N, D] (its 4 heads' contribution through
the output projection) in bf16; the host sums the 4 partials per batch and
adds b_out.

v2 schedule: the attention stream (S^T matmul + exp + PV) is ScalarE-bound
(~81us of exp at 1 elem/cycle/lane vs ~44us of PE work), so the projection
matmuls (~55us of PE work) are NOT done in a separate phase: they are emitted
as PE filler thunks interleaved into the attention units, keeping the PE busy
(HAM clock at 2.4 GHz) while ScalarE grinds through the exps.

Per-core layout:
  xT   [1024, 2048]  (host-pretransposed x[b]), loaded per (e,nq) chunk
  QT/KT stored transposed [dk, n] as head-pair tiles [128, 2048]
  V    stored natural as [128(keys), 16 blocks, 4 heads, 65] with a ones
       column appended (col 64) so P@V' also yields the softmax denominator.
  S^T  per (head-pair, q-tile 512, key-block 128) as [128, 2, 512] in PSUM:
       matmul(lhsT=KT slice [64,128], rhs=QT slice [64,512]); the two heads
       sit at base partitions 0/64 so their matmuls row-tile and run
       concurrently on the PE. Causal masking = additive -30000 on PSUM
       (DVE); padding mask is a per-key bias fused into the exp; one
       exp(0.125*s + bias) on ScalarE writes P^T straight to SBUF as bf16.
  ctx'^T [65, 512] accumulated in PSUM over key blocks; PV matmuls are
       emitted one unit behind their exps (riffled into the next unit's S
       stream) so the in-order PE never drains waiting on ScalarE.
  Normalization (one unit behind PV): ctx PSUM rows are copied out on DVE
       immediately after the last PV (so the 2 ctx PSUM banks recycle
       without waiting on the softmax-denominator reciprocal), then
       r = reciprocal_approx_fast(denom row) (single custom-DVE op, ~5x
       faster than the iterative divide), partition-broadcast (GpSimd),
       one DVE multiply into the bf16 ctx buffer.
  Out projection per q-tile once both head-pairs are normalized:
       matmul(lhsT=ctxT [128,128], rhs=WoutT [128,512]) acc over the two
       head-pair chunks, cast bf16, DMA out.

All matmul operands are bf16 (pre-rounded on host for the inputs; on-device
casts for intermediates); accumulation is fp32 in PSUM, and the softmax /
masking / normalization arithmetic is fp32.
"""

import math
import os

import numpy as np

B, N, D, H = 2, 2048, 1024, 16
DK = D // H  # 64
NCORES = 8
HEADS_PER_CORE = 4
QTILE = 512
KBLK = 128
NEG = -30000.0
NEGB = -3750.0  # pad bias applied after the 0.125 scale inside exp
SCALE = 1.0 / math.sqrt(float(DK))  # 0.125

# Set by run() when tracing is enabled (test.py reads this).
LAST_RESULTS = None


def _build_program(kb_max: int, jpad_min: int):
    import concourse.tile as tile
    from concourse import bacc, mybir

    F32 = mybir.dt.float32
    BF16 = mybir.dt.bfloat16
    EXP = mybir.ActivationFunctionType.Exp
    ADD = mybir.AluOpType.add
    MULT = mybir.AluOpType.mult

    nc = bacc.Bacc(None)

    xt_d = nc.dram_tensor("xt", [D, N], BF16, kind="ExternalInput")
    wqkv_d = nc.dram_tensor("wqkv", [D, 768], BF16, kind="ExternalInput")
    wout_d = nc.dram_tensor("wout", [256, D], BF16, kind="ExternalInput")
    ptri_d = nc.dram_tensor("ptri", [128, 912], F32, kind="ExternalInput")
    ones_d = nc.dram_tensor("ones65", [128, 64], BF16, kind="ExternalInput")
    out_d = nc.dram_tensor("out", [N, D], BF16, kind="ExternalOutput")

    NB = N // KBLK  # 16 key/row blocks
    NQT = N // QTILE  # 4 q tiles
    nb_used = min(NB, kb_max)

    with tile.TileContext(nc) as tc:
        with (
            tc.tile_pool(name="w", bufs=1) as w_pool,
            tc.tile_pool(name="big", bufs=1) as big_pool,
            tc.tile_pool(name="craw", bufs=5) as craw_pool,
            tc.tile_pool(name="work", bufs=2) as work_pool,
            tc.tile_pool(name="osb", bufs=7) as osb_pool,
            tc.tile_pool(name="pt", bufs=26) as pt_pool,
            tc.tile_pool(name="ps_main", bufs=2, space="PSUM") as ps_main,
            tc.tile_pool(name="ps_aux", bufs=2, space="PSUM") as ps_aux,
            tc.tile_pool(name="ps_ctx", bufs=1, space="PSUM") as ps_ctx,
        ):
            # ---- input DMAs (DMA queues run independently of engines) ----
            wqkv_t = w_pool.tile([128, 8, 768], BF16, tag="wqkv")
            wo_t = w_pool.tile([128, 2, D], BF16, tag="wo")
            nc.sync.dma_start(
                wqkv_t[:], wqkv_d[:].rearrange("(e p) m -> p e m", p=128)
            )
            # denominator staging: one tile per reciprocal batch (the DVE
            # Reciprocal and the batch source must start at partition 0)
            stages = [
                w_pool.tile([8, 512], F32, tag="stageA", name="stageA"),
                w_pool.tile([4, 512], F32, tag="stageB", name="stageB"),
                w_pool.tile([4, 512], F32, tag="stageC", name="stageC"),
            ]
            rstages = [
                w_pool.tile([8, 512], F32, tag="rstageA", name="rstageA"),
                w_pool.tile([4, 512], F32, tag="rstageB", name="rstageB"),
                w_pool.tile([4, 512], F32, tag="rstageC", name="rstageC"),
            ]

            def stage_slot(uidx, hh):
                if uidx < 4:
                    return 0, 2 * uidx + hh
                if uidx < 6:
                    return 1, 2 * (uidx - 4) + hh
                return 2, 2 * (uidx - 6) + hh

            ptri_t = w_pool.tile([128, 912], F32, tag="ptri")
            padb_t = ptri_t[:, 0:16]
            trineg_t = ptri_t[:, 16:912]
            # xT chunks: [e, nq] -> [128, 512]; issued in the order the
            # units consume them (q-tile 0, then 3, then 2 — the V blocks
            # 8-15 riffled during unit 3 read q-tile-2 columns — then 1).  Issues
            # are spread across the Sync / Scalar / Tensor engine queues:
            # each issue costs ~0.6us on its engine and each HW queue only
            # moves ~21 GB/s, so both issue serialization and queue depth
            # matter.  Scalar and Tensor are idle during the lead-in.
            xt = [big_pool.tile([128, N], BF16, tag=f"xt{e}", name=f"xt{e}") for e in range(8)]
            for e in range(8):
                nc.sync.dma_start(
                    xt[e][:, 0:512], xt_d[e * 128:(e + 1) * 128, 0:512]
                )
            nc.sync.dma_start(ptri_t[:], ptri_d[:])
            for nq in (3, 2, 1):
                for e in range(8):
                    nc.sync.dma_start(
                        xt[e][:, nq * 512:(nq + 1) * 512],
                        xt_d[e * 128:(e + 1) * 128, nq * 512:(nq + 1) * 512],
                    )
            nc.sync.dma_start(wo_t[:], wout_d[:].rearrange("(c p) m -> p c m", p=128))

            # V' tile: [keys 128, key-block 16, head 4, 65]; col 64 <- ones
            v4 = big_pool.tile([128, NB, 4, 65], BF16, tag="v4")
            nc.sync.dma_start(
                v4[:, :, :, 64:65],
                ones_d[:].rearrange("p (b h o) -> p b h o", h=4, o=1),
            )

            qt_pair = [big_pool.tile([128, N], BF16, tag=f"qt{p}", name=f"qt{p}") for p in range(2)]
            kt_pair = [big_pool.tile([128, N], BF16, tag=f"kt{p}", name=f"kt{p}") for p in range(2)]
            ctx_pair = [big_pool.tile([128, N], BF16, tag=f"ctx{p}", name=f"ctx{p}") for p in range(2)]

            # ---- projection thunks (PE filler during the attention stream) --
            def emit_qk(name, pair, nq):
                wofs = 0 if name == "q" else 256
                dst = qt_pair[pair] if name == "q" else kt_pair[pair]
                ps = ps_aux.tile([128, 512], F32, tag="aux")
                for e in range(8):
                    nc.tensor.matmul(
                        ps[:],
                        wqkv_t[:, e, wofs + pair * 128:wofs + (pair + 1) * 128],
                        xt[e][:, nq * 512:(nq + 1) * 512],
                        start=(e == 0),
                        stop=(e == 7),
                    )
                nc.vector.tensor_copy(dst[:, nq * 512:(nq + 1) * 512], ps[:])

            def emit_v(m):
                # V natural for key blocks {2m, 2m+1}: [keys, 4*64]
                ps = ps_aux.tile([128, 512], F32, tag="aux")
                for h in range(2):
                    nb = 2 * m + h
                    if nb >= nb_used:
                        continue
                    for e in range(8):
                        nc.tensor.matmul(
                            ps[:, h * 256:(h + 1) * 256],
                            xt[e][:, nb * 128:(nb + 1) * 128],
                            wqkv_t[:, e, 512:768],
                            start=(e == 0),
                            stop=(e == 7),
                        )
                hi = min(2 * m + 2, nb_used)
                nc.vector.tensor_copy(
                    v4[:, 2 * m:hi, :, 0:64],
                    ps[:, 0:(hi - 2 * m) * 256].rearrange(
                        "p (b h d) -> p b h d", h=4, d=64
                    ),
                )

            # ---- attention units ----
            # A unit is (head-pair, q-tile). Ordered so that ScalarE-light
            # units (qt0) come first (lead-in) and heavy/medium alternate.
            units = []
            for pair, qt in [(0, 0), (1, 0), (0, 3), (1, 3),
                             (0, 1), (1, 1), (0, 2), (1, 2)]:
                units.append((pair, qt, min(4 * qt + 4, kb_max)))

            def nqs_needed(qt):
                return min(qt + 1, (kb_max + 3) // 4)

            # filler thunks per unit index (emitted inside the unit's j-loop)
            fillers = [[] for _ in range(9)]
            # prelude (before unit 0): its own QT/KT
            fillers[0] = [lambda: emit_qk("q", 0, 0), lambda: emit_qk("k", 0, 0)]
            fillers[1] = [lambda: emit_qk("q", 1, 0), lambda: emit_qk("k", 1, 0),
                          lambda: emit_v(0), lambda: emit_v(1)]
            fillers[2] = [lambda: emit_qk("q", 0, 3)] + [
                (lambda nq: lambda: emit_qk("k", 0, nq))(nq)
                for nq in range(1, nqs_needed(3))
            ]
            fillers[3] = [lambda: emit_qk("q", 1, 3)] + [
                (lambda nq: lambda: emit_qk("k", 1, nq))(nq)
                for nq in range(1, nqs_needed(3))
            ] + [(lambda m: lambda: emit_v(m))(m)
                 for m in range(2, (nb_used + 1) // 2)]
            fillers[4] = [lambda: emit_qk("q", 0, 1)]
            fillers[5] = [lambda: emit_qk("q", 1, 1)]
            fillers[6] = [lambda: emit_qk("q", 0, 2)]
            fillers[7] = [lambda: emit_qk("q", 1, 2)]
            # fillers[k] emitted during unit k-1's loop (shift by one):
            # fillers[0] is the prelude, emitted before the loop.

            def emit_st_exp(pair, qt, nchunks, prev, fill):
                """S^T + mask + exp for both heads, with the previous unit's
                PV matmuls riffled in and projection/outproj filler thunks
                spread across the j-loop."""
                if prev is None:
                    ppv = []
                else:
                    ppair, pqt, pn, ppv, pctx2 = prev

                def rif(k):
                    while ppv and ppv[0][0] <= k:
                        jj, ptt, poff = ppv.pop(0)
                        for hh in range(2):
                            nc.tensor.matmul(
                                pctx2[hh][:, poff:],
                                v4[:, jj, 2 * ppair + hh, :],
                                ptt[:, hh, poff:],
                                start=(jj == 0),
                                stop=(jj == pn - 1),
                                skip_group_check=True,
                            )

                nfill = len(fill)
                fill_at = [
                    (i * nchunks) // nfill if nfill else 0 for i in range(nfill)
                ]
                fi = 0

                pv = []
                for j in range(nchunks):
                    rif(j)
                    d = j - 4 * qt
                    # exact-causal column trim (keep matmul N >= 256)
                    off = 128 * d if d >= 1 else 0
                    st_ps = ps_main.tile([128, 2, 512], F32, tag="blk")
                    for hh in range(2):
                        hp = slice(64 * hh, 64 * hh + 64)
                        nc.tensor.matmul(
                            st_ps[:, hh, off:],
                            kt_pair[pair][hp, j * 128:(j + 1) * 128],
                            qt_pair[pair][hp, qt * 512 + off:(qt + 1) * 512],
                            start=True,
                            stop=True,
                        )
                    if d >= 0:
                        # causal add -30000; with off = 128*d the masked
                        # triangle lies entirely in cols [off, off+128)
                        u0 = 384 - 128 * d + off
                        w = min(128, 512 - off)
                        for hh in range(2):
                            nc.vector.tensor_tensor(
                                st_ps[:, hh, off:off + w],
                                st_ps[:, hh, off:off + w],
                                trineg_t[:, u0:u0 + w],
                                ADD,
                            )
                    pt_t = pt_pool.tile([128, 2, 512], BF16, tag="pt")
                    kw = {}
                    if j >= jpad_min:  # per-key pad bias (same for both heads)
                        kw["bias"] = padb_t[:, j:j + 1]
                    nc.scalar.activation(
                        pt_t[:, :, off:], st_ps[:, :, off:], EXP, scale=SCALE, **kw
                    )
                    pv.append((j, pt_t, off))
                    while fi < nfill and fill_at[fi] <= j:
                        fill[fi]()
                        fi += 1
                rif(10 ** 9)
                while fi < nfill:
                    fill[fi]()
                    fi += 1
                return pv

            def emit_pv(pair, qt, nchunks, pv, ctx2):
                for j, pt_t, off in pv:
                    for hh in range(2):
                        nc.tensor.matmul(
                            ctx2[hh][:, off:],
                            v4[:, j, 2 * pair + hh, :],
                            pt_t[:, hh, off:],
                            start=(j == 0),
                            stop=(j == nchunks - 1),
                            skip_group_check=True,
                        )

            def emit_copyout(uidx, ctx2):
                # move ctx (incl denominator row) out of PSUM right away so
                # the 2 ctx banks recycle without waiting on normalization;
                # denominator rows are DMA'd onto consecutive partitions of
                # `stage` so one batched reciprocal covers many units.
                craws = []
                for hh in range(2):
                    craw = craw_pool.tile([65, 512], F32, tag=f"craw{hh}", name=f"craw{hh}")
                    nc.vector.tensor_copy(craw[:], ctx2[hh][:])
                    bi, r = stage_slot(uidx, hh)
                    nc.sync.dma_start(stages[bi][r:r + 1, :], craw[64:65, :])
                    craws.append(craw)
                return craws

            def emit_recip(bi):
                nc.vector.reciprocal(rstages[bi][:], stages[bi][:])

            def emit_normalize(uidx, pair, qt, craws, hh):
                hp = slice(64 * hh, 64 * hh + 64)
                craw = craws[hh]
                bi, r = stage_slot(uidx, hh)
                rrow = work_pool.tile([1, 512], F32, tag="rrow")
                nc.sync.dma_start(rrow[:], rstages[bi][r:r + 1, :])
                rbr = work_pool.tile([64, 512], F32, tag="rbr")
                nc.gpsimd.partition_broadcast(rbr[:], rrow[:])
                nc.vector.tensor_tensor(
                    ctx_pair[pair][hp, qt * 512:(qt + 1) * 512],
                    craw[0:64, :],
                    rbr[:],
                    MULT,
                )

            deferred_osb = []

            def emit_outproj_nb(nb, tail=False, defer_dma=False):
                # output projection for one 128-row n-block
                osb = osb_pool.tile([128, D], BF16, tag="osb")
                for fc in range(2):
                    ps = ps_aux.tile([128, 512], F32, tag="aux")
                    for pr2 in range(2):
                        nc.tensor.matmul(
                            ps[:],
                            ctx_pair[pr2][:, nb * 128:(nb + 1) * 128],
                            wo_t[:, pr2, fc * 512:(fc + 1) * 512],
                            start=(pr2 == 0),
                            stop=(pr2 == 1),
                        )
                    if tail:
                        # ScalarE is idle in the tail; DVE is not
                        nc.scalar.copy(osb[:, fc * 512:(fc + 1) * 512], ps[:])
                    else:
                        nc.vector.tensor_copy(
                            osb[:, fc * 512:(fc + 1) * 512], ps[:]
                        )
                if defer_dma:
                    # queue the bulky output DMA later so the tail's tiny
                    # stage/rrow DMAs don't sit behind it on the sync queues
                    deferred_osb.append((nb, osb))
                else:
                    nc.sync.dma_start(out_d[nb * 128:(nb + 1) * 128, :], osb[:])

            # ---- main loop ----
            # per unit k: st_exp(k) [rifles PV(k-1) + filler thunks],
            # copyout(k-1).  Normalization runs in 3 static batches (one
            # batched reciprocal each) riffled into later units' loops:
            #   during u5: recip rows 0:8, norm u0/u1, outproj qt0
            #   during u6: norm u2/u3, outproj qt3
            #   during u7: recip rows 8:12, norm u4/u5, outproj qt1
            #   tail:      recip rows 12:16, norm u6/u7, outproj qt2
            craws_by_unit = {}

            def norm_thunks(uidx):
                pair, qt, _ = units[uidx]
                return [
                    (lambda hh: lambda: emit_normalize(
                        uidx, pair, qt, craws_by_unit[uidx], hh))(hh)
                    for hh in range(2)
                ]

            def op_thunks(nbs, tail=False, defer_dma=False):
                return [
                    (lambda nb: lambda: emit_outproj_nb(
                        nb, tail=tail, defer_dma=defer_dma))(nb)
                    for nb in nbs
                ]

            fillers[6] += ([lambda: emit_recip(0)] + norm_thunks(0)
                           + norm_thunks(1) + op_thunks(range(0, 4)))
            fillers[7] += (norm_thunks(2) + norm_thunks(3)
                           + op_thunks(range(12, 16)))
            fillers[8] += ([lambda: emit_recip(1)] + norm_thunks(4)
                           + norm_thunks(5) + op_thunks(range(4, 8)))

            prev_pv = None
            for k, (pair, qt, nchunks) in enumerate(units):
                if k == 0:
                    # prelude projections must precede the first S matmul
                    for th in fillers[0]:
                        th()
                pv = emit_st_exp(pair, qt, nchunks, prev_pv, fillers[k + 1])
                if prev_pv is not None:
                    craws_by_unit[k - 1] = emit_copyout(k - 1, prev_pv[4])
                ctx2 = [
                    ps_ctx.tile([65, 512], F32, tag=f"ctx{hh}", name=f"ctx{hh}")
                    for hh in range(2)
                ]
                prev_pv = (pair, qt, nchunks, pv, ctx2)

            # ---- tail ----
            ppair, pqt, pn, ppv, pctx2 = prev_pv
            emit_pv(ppair, pqt, pn, ppv, pctx2)
            craws_by_unit[7] = emit_copyout(7, pctx2)
            emit_recip(2)
            for uidx in (6, 7):
                pair, qt, _ = units[uidx]
                for hh in range(2):
                    emit_normalize(uidx, pair, qt, craws_by_unit[uidx], hh)
            for nb, osb in deferred_osb:
                nc.sync.dma_start(out_d[nb * 128:(nb + 1) * 128, :], osb[:])
            deferred_osb.clear()
            for nb in range(8, 12):
                emit_outproj_nb(nb, tail=True)

    nc.compile()
    return nc


_PROGRAM_CACHE = {}


def kernel(x, attention_mask, W_Q, W_K, W_V, W_out, b_out):
    global LAST_RESULTS
    from concourse.bass_utils import run_bass_kernel_spmd

    x = np.ascontiguousarray(x, dtype=np.float32)
    attention_mask = np.asarray(attention_mask)
    lengths = attention_mask.astype(np.int64).sum(axis=1)
    kb_max = int(math.ceil(lengths.max() / KBLK))
    jpad_min = int(lengths.min() // KBLK)

    key = (kb_max, jpad_min)
    if key not in _PROGRAM_CACHE:
        _PROGRAM_CACHE[key] = _build_program(kb_max, jpad_min)
    nc = _PROGRAM_CACHE[key]

    # host-side input prep (matmul operands pre-cast to bf16)
    import ml_dtypes
    BF = ml_dtypes.bfloat16
    xT = [np.ascontiguousarray(x[b].T.astype(BF)) for b in range(B)]
    wqT = np.asarray(W_Q, dtype=np.float32).T.astype(BF)
    wkT = np.asarray(W_K, dtype=np.float32).T.astype(BF)
    wvT = np.asarray(W_V, dtype=np.float32).T.astype(BF)
    woT = np.ascontiguousarray(np.asarray(W_out, dtype=np.float32).T.astype(BF))
    # padbias[p, j] = 0 if key j*128+p is real else -3750; fused with trineg
    # (trineg[p, u] = NEG if u < p + 384 else 0) into one [128, 912] input.
    pp = np.arange(128)[:, None]
    uu = np.arange(896)[None, :]
    trineg = np.where(uu < pp + 384, NEG, 0.0).astype(np.float32)
    ptri = []
    for b in range(B):
        padb = np.where(
            attention_mask[b].reshape(16, 128).T != 0, 0.0, NEGB
        ).astype(np.float32)
        ptri.append(np.ascontiguousarray(np.concatenate([padb, trineg], axis=1)))
    ones65 = np.ones((128, 64), dtype=BF)

    in_maps = []
    for c in range(NCORES):
        b, g = divmod(c, 4)
        sl = slice(g * 256, (g + 1) * 256)
        wqkv = np.ascontiguousarray(
            np.concatenate([wqT[:, sl], wkT[:, sl], wvT[:, sl]], axis=1)
        )
        in_maps.append(
            {
                "xt": xT[b],
                "wqkv": wqkv,
                "wout": np.ascontiguousarray(woT[sl, :]),
                "ptri": ptri[b],
                "ones65": ones65,
            }
        )

    trace = bool(int(os.environ.get("KERNEL_TRACE", "0")))
    ncores_run = int(os.environ.get("KERNEL_NCORES", str(NCORES)))
    res = run_bass_kernel_spmd(
        nc,
        in_maps[:ncores_run],
        core_ids=list(range(ncores_run)),
        trace=trace,
        trace_cores=list(range(ncores_run)) if trace else None,
    )
    LAST_RESULTS = res

    out = np.zeros((B, N, D), dtype=np.float32)
    for c in range(len(res.results)):
        out[c // 4] += np.asarray(res.results[c]["out"], dtype=np.float32)
    out += np.asarray(b_out, dtype=np.float32)[None, None, :]
    return out
